# revision 20
# baseline (speedup 1.0000x reference)
"""Additive (Bahdanau) attention scores on 8 Trainium2 NeuronCores.

scores[b,h,q,k] = sum_d V[d]*tanh((Q@W1+b1)[b,h,q,d] + (K@W2+b2)[b,h,k,d]) + bV

Strategy: tanh(x) on the observed argument range is approximated by a
J=3-term free-frequency sine sum fitted with a Gaussian(|x|) weight
(end-to-end rel err ~4e-3, well under the 2e-2 gate):
    tanh(x) ~= sum_j AL[j]*sin(OM[j]*x)
sin(w*(a+b)) separates: sin(wa+p1)cos(wb+p2) + cos(wa+p1)sin(wb+p2)
with p1+p2 = 0.  With fp16 atoms (rep 0/1 in partition halves)
    A_j[(rep,d), q] = [sin(w_j a_qd + w_j b1_d); cos(...)]
    B_j[(rep,d), k] = AL_j V_d [cos(w_j b_kd + w_j b2_d); sin(...)]
scores = sum_j A_j^T B_j + bV: J accumulating 128-contraction matmuls
per 128x512 output tile on the PE (fp32 psum).

The scalar engine's Sin only accepts [-pi, pi]; out-of-range atoms are
range-reduced in integer turns.  The affine+cast for the A side runs ON
THE SCALAR ENGINE as an Identity activation with int32 output reading
the projection PSUM directly (no PSUM->SBUF copy needed):
    yA = int32(Ident(aT2*s_j + cA))        [ACT, PSUM src]
    yB = int32(u_b*s_j + cB)               [DVE, from the B-side copy]
    m  = y & 0x3FFFF  (per side, 2x mode)  [DVE]
    atom = Sin((2pi/2^18)*m - pi)          [ACT, merged A|B, f16 out]
j=0 fits [-pi, pi] and is two direct Sins straight from PSUM.  GpSimd
only does warm-up work (it is ~2.2x slower per element with ~750ns
pipeline drains).

Performance structure (trace-driven):
  - 2 input DMAs (consts+head0 first) so projections start early
  - dummy+filler matmuls keep the PE continuously busy: the tensor
    engine needs ~3us of back-to-back work to ramp 1.2 -> 2.4 GHz
  - score matmuls run j-outer over 4 PSUM pair-tiles (2 banks each);
    a finished pair is drained in ONE op (fp32->fp16, +bV) and fed to
    its own output DMA queue
  - output is fp16 (halves DMA bytes; adds ~2e-4 rel err)

Sharding: data-parallel over the 16 (b,h) pairs, 2 per core.
"""

import sys

for _p in ("/opt/trn_rl_repo",):
    if _p not in sys.path:
        sys.path.insert(0, _p)

import numpy as np

import concourse.bass as bass
import concourse.tile as tile
from concourse.tile import add_dep_helper
from concourse import mybir
from concourse.bass_utils import run_bass_kernel_spmd

# Free-frequency sine fits of tanh, least-squares with weight
# sqrt(exp(-x^2/2/0.83^2) + 3e-3) on [0, R]: R -> (omegas, alphas).
FITS = {
    5.2: (
        np.array([0.48444192, 1.49028523, 2.72139567]),
        np.array([1.16681294, 0.20458018, 0.04279165]),
    ),
}

N_CORES = 8
HPC = 2          # (b*h) heads per core: 16 / 8
LQ = 512
LK = 512
D = 64
QT = LQ // 128   # q tiles per head
NT = HPC * QT    # output tiles per core
NP = NT // 2     # output pair-tiles
TWO_PI = 2.0 * np.pi
MARGIN = 0.02    # stay this far inside [-pi, pi] for direct (no-mod) atoms
FSC = 262144.0   # 2^18 phase quantization

# input blocks: [consts, h0 data x2, h1 data x2]
NBLK = 1 + HPC * (LQ // 256)
BLK_CC = 0
CC0 = 64         # consts col offsets: cc A cols CC0..CC0+J-1, B next J
N_DUMMY = 4      # PE warm-up matmuls during input DMA
N_FILL = 10      # PE filler matmuls bridging projections -> first scores


def _plan(b1, b2, u_bound_a, u_bound_b, R_need):
    """Per-atom constants. Returns (om, al, J, cc[128, 2J], plan[2J])
    where plan[col] = ("direct", omega) or ("fold", omega)."""
    Rs = sorted(FITS.keys())
    R_fit = None
    for r in Rs:
        if r >= R_need:
            R_fit = r
            break
    if R_fit is None:
        R_fit = Rs[-1]
    om, al = FITS[R_fit]
    J = len(om)

    b1d = np.concatenate([b1, b1]).astype(np.float64)
    b2d = np.concatenate([b2, b2]).astype(np.float64)
    phaseA = np.concatenate([np.zeros(64), np.full(64, np.pi / 2)])
    phaseB = np.concatenate([np.full(64, np.pi / 2), np.zeros(64)])

    kinds = []
    for j in range(J):
        w = float(om[j])
        okA = w * u_bound_a + np.abs(w * b1d + phaseA).max() <= np.pi - MARGIN
        okB = w * u_bound_b + np.abs(w * b2d + phaseB).max() <= np.pi - MARGIN
        kinds.append("direct" if (okA and okB) else "fold")

    cc = np.empty((128, 2 * J), np.float32)
    plan = []
    for col in range(2 * J):
        j = col % J
        w = float(om[j])
        c = (w * b1d + phaseA) if col < J else (w * b2d + phaseB)
        if kinds[j] == "direct":
            cc[:, col] = c.astype(np.float32)
        else:
            cc[:, col] = ((c / TWO_PI + 0.5) * FSC).astype(np.float32)
        plan.append((kinds[j], w))
    return om, al, J, cc, plan


def build_nc(bV_val, J, plan):
    f32 = mybir.dt.float32
    f16 = mybir.dt.float16
    i32 = mybir.dt.int32
    SIN = mybir.ActivationFunctionType.Sin
    IDENT = mybir.ActivationFunctionType.Identity

    NEGPI = CC0 + 2 * J
    BVCOL = NEGPI + 1
    VC0 = BVCOL + 1
    NA = HPC * LQ  # atom columns per side
    directs = [j for j in range(J) if plan[j][0] == "direct"]
    folds = [j for j in range(J) if plan[j][0] == "fold"]
    jorder = directs + folds

    nc = bass.Bass()
    # qk: [128, NBLK, 128] f32. Block 0 = consts (W1dup/W2dup fp16 in
    # cols 0:64, cc/negpi/bV/vcoef in cols 64+). Blocks 1..4: partitions
    # 0:64 = Q^T fp16 tile, 64:128 = K^T (host pre-transposed).
    qk = nc.declare_dram_parameter("qk", [128, NBLK, 128], f32, isOutput=False)
    # out[h, pair, p, s, k] = scores[h, (2*pair+s)*128+p, k] in fp16
    out = nc.declare_dram_parameter("out", [HPC, QT // 2, 128, 2, LK], f16,
                                    isOutput=True)

    with tile.TileContext(nc) as tc:
        spsum_cm = tc.tile_pool(name="spsum", bufs=2, space="PSUM")
        spsum = spsum_cm.__enter__()
        ppsum_cm = tc.tile_pool(name="ppsum", bufs=1, space="PSUM")
        ppsum = ppsum_cm.__enter__()
        with (
            tc.tile_pool(name="inp", bufs=1) as inp,
            tc.tile_pool(name="proj", bufs=1) as proj_pool,
            tc.tile_pool(name="marg", bufs=2 * max(1, len(folds))) as marg_pool,
            tc.tile_pool(name="mm", bufs=max(1, len(folds))) as mm_pool,
            tc.tile_pool(name="atoms", bufs=J + len(directs)) as atom_pool,
            tc.tile_pool(name="bsc", bufs=J) as bsc_pool,
            tc.tile_pool(name="sout", bufs=NP) as sout_pool,
        ):
            insts = {"PE": [], "ACT": [], "DVE": [], "POOL": [], "DMA": []}
            qk_sb = inp.tile([128, NBLK, 128], f32)
            # consts + head-0 data first, head-1 data on a second queue
            insts["DMA"].append(nc.sync.dma_start(
                out=qk_sb[:, 0:3, :], in_=qk[:, 0:3, :]))
            insts["DMA"].append(nc.sync.dma_start(
                out=qk_sb[:, 3:5, :], in_=qk[:, 3:5, :]))

            # Warm-up touches: one tiny instruction per engine that reads
            # the first DMA's payload, so each engine observes that
            # semaphore early and later instructions carry at most ONE
            # new wait.  The ACT warm-up is a Sin so the activation table
            # set loads during the input DMA, off the critical path.
            warm = inp.tile([128, 4], f32, tag="warm")
            insts["POOL"].append(
                nc.gpsimd.tensor_copy(warm[:, 0:1], qk_sb[:, BLK_CC, 0:1]))
            insts["DVE"].append(
                nc.vector.tensor_copy(warm[:, 1:2], qk_sb[:, BLK_CC, 0:1]))
            insts["ACT"].append(
                nc.scalar.activation(warm[:, 2:3],
                                     qk_sb[:, BLK_CC, NEGPI:NEGPI + 1],
                                     SIN, bias=0.0, scale=0.25))

            # PE warm-up: the tensor engine needs ~3us of continuous work
            # to ramp to full clock.  Run dummy matmuls on a memset
            # scratch tile while the input DMA is in flight.
            scratch = inp.tile([128, 256], f32, tag="scratch")
            insts["POOL"].append(nc.gpsimd.memset(scratch, 0))
            # warm-up matmuls write score-pair-0's bank; the first real
            # score matmul re-initializes it with start=True
            pair0 = spsum.tile([128, 2, LK], f32, tag="spair")
            dummy_ps = pair0[:, 0, :]
            dlhs = scratch[:, 0:64].bitcast(f16)
            drhs = scratch[:, 0:256].bitcast(f16)
            for _ in range(N_DUMMY):
                insts["PE"].append(nc.tensor.matmul(
                    dummy_ps, lhsT=dlhs, rhs=drhs, start=True, stop=True))
            # absorbs the first input-DMA semaphore on the PE
            insts["PE"].append(nc.tensor.matmul(
                dummy_ps[:, 0:128],
                lhsT=qk_sb[0:64, BLK_CC, 0:64].bitcast(f16),
                rhs=qk_sb[0:64, BLK_CC, 0:64].bitcast(f16),
                start=True, stop=True))

            # Projections: contract straight out of the input tile.
            # W2dup sits in partitions 64:128 of the consts block so
            # lhsT/rhs partition bases match.
            aT2 = ppsum.tile([128, HPC, LQ], f32, tag="aT2")
            bT2 = ppsum.tile([128, HPC, LK], f32, tag="bT2")
            nb = LQ // 256
            for h in range(HPC):
                insts["PE"].append(nc.tensor.matmul(
                    aT2[:, h, :],
                    lhsT=qk_sb[0:64, BLK_CC, 0:64].bitcast(f16),
                    rhs=qk_sb[0:64, 1 + h * nb:1 + (h + 1) * nb, :].bitcast(f16),
                    start=True, stop=True))
                insts["PE"].append(nc.tensor.matmul(
                    bT2[:, h, :],
                    lhsT=qk_sb[64:128, BLK_CC, 0:64].bitcast(f16),
                    rhs=qk_sb[64:128, 1 + h * nb:1 + (h + 1) * nb, :].bitcast(f16),
                    start=True, stop=True))
            # Fillers: keep the PE clock ramped while the first atoms
            # are produced.
            for _ in range(N_FILL):
                insts["PE"].append(nc.tensor.matmul(
                    dummy_ps, lhsT=dlhs, rhs=drhs, start=True, stop=True))

            aT2f = aT2[:, :, :]  # [128, HPC, LQ] APs; ops treat free dims flat
            bT2f = bT2[:, :, :]

            negpi = qk_sb[:, BLK_CC, NEGPI:NEGPI + 1]
            bvcol = qk_sb[:, BLK_CC, BVCOL:BVCOL + 1]

            def ccv(col):
                return qk_sb[:, BLK_CC, CC0 + col:CC0 + col + 1]

            # ---- atom production ----
            # Engine orders (all WAR waits on the later PSUM-bank reuse
            # collapse by implication: aT2 is read by ACT only, bT2 by
            # DVE only, and the first spsum2 matmul already waits on
            # later ACT/DVE ops):
            #   ACT: fcA per fold, SA_j/SB_j directs (SB from u_b),
            #        merged Sin per fold, 2 pair drains
            #   DVE: copyB, fold/AND per fold, B scales, 2 pair drains
            atomsA = {}
            braw = {}
            atomsB = {}
            u_b = proj_pool.tile([128, NA], f32, tag="u_b")
            ys = {}
            for j in folds:
                yA = marg_pool.tile([128, NA], i32, tag=f"yA{j}", name=f"yA{j}")
                # per-head halves: the first one starts as soon as the
                # h0 A-projection lands, filling the ACT idle window
                for h in range(HPC):
                    insts["ACT"].append(nc.scalar.activation(
                        yA[:, h * LQ:(h + 1) * LQ], aT2[:, h, :], IDENT,
                        bias=ccv(j), scale=float(FSC * plan[j][1] / TWO_PI)))
                ys[j] = yA
            for j in directs:
                aA = atom_pool.tile([128, NA], f16, tag=f"dirA{j}", name=f"dirA{j}")
                insts["ACT"].append(nc.scalar.activation(
                    aA, aT2f, SIN, bias=ccv(j), scale=float(plan[j][1])))
                atomsA[j] = aA
            for h in range(HPC):
                insts["DVE"].append(nc.vector.tensor_copy(
                    u_b[:, h * LK:(h + 1) * LK], bT2[:, h, :]))
            for j in directs:
                bR = atom_pool.tile([128, NA], f16, tag=f"dirB{j}", name=f"dirB{j}")
                insts["ACT"].append(nc.scalar.activation(
                    bR, u_b, SIN, bias=ccv(J + j), scale=float(plan[J + j][1])))
                braw[j] = bR

            def emit_mul(j):
                aB = bsc_pool.tile([128, NA], f16, tag=f"atomB{j}",
                                   name=f"atomB{j}")
                insts["DVE"].append(nc.vector.tensor_scalar_mul(
                    aB, braw[j], qk_sb[:, BLK_CC, VC0 + j:VC0 + j + 1]))
                atomsB[j] = aB

            muls_pending = list(directs)
            for i, j in enumerate(folds):
                yB = marg_pool.tile([128, NA], i32, tag=f"yB{j}", name=f"yB{j}")
                insts["DVE"].append(nc.vector.tensor_scalar(
                    out=yB, in0=u_b, scalar1=float(FSC * plan[J + j][1] / TWO_PI),
                    scalar2=ccv(J + j),
                    op0=mybir.AluOpType.mult, op1=mybir.AluOpType.add))
                m = mm_pool.tile([128, 2 * NA], i32, tag=f"m{j}", name=f"m{j}")
                insts["DVE"].append(nc.vector.tensor_scalar(
                    out=m[:, 0:NA], in0=ys[j], scalar1=0x3FFFF, scalar2=None,
                    op0=mybir.AluOpType.bitwise_and))
                insts["DVE"].append(nc.vector.tensor_scalar(
                    out=m[:, NA:2 * NA], in0=yB, scalar1=0x3FFFF, scalar2=None,
                    op0=mybir.AluOpType.bitwise_and))
                sAB = atom_pool.tile([128, 2 * NA], f16, tag=f"sAB{j}",
                                     name=f"sAB{j}")
                insts["ACT"].append(nc.scalar.activation(
                    sAB, m, SIN, bias=negpi, scale=float(TWO_PI / FSC)))
                atomsA[j] = sAB[:, 0:NA]
                braw[j] = sAB[:, NA:2 * NA]
                # direct-j B scales slot in after the first fold round
                # (their ACT dep is ready by then; no DVE stall)
                if i == 0:
                    for jd in muls_pending:
                        emit_mul(jd)
                    muls_pending = []
            for jd in muls_pending:
                emit_mul(jd)
            for j in folds:
                emit_mul(j)

            ppsum_cm.__exit__(None, None, None)
            spsum2_cm = tc.tile_pool(name="spsum2", bufs=NP - 2, space="PSUM")
            spsum2 = spsum2_cm.__enter__()

            # Score matmuls, j-outer over 4 pair-tiles (2 banks each).
            pair1 = spsum.tile([128, 2, LK], f32, tag="spair")
            pairs = [pair0, pair1]
            for p in range(2, NP):
                pt = spsum2.tile([128, 2, LK], f32, tag="spair2")
                pairs.append(pt)
            last_order = [0, 1, 2, 3, 6, 7, 4, 5]
            for idx, j in enumerate(jorder):
                torder = last_order if idx == J - 1 else range(NT)
                for t in torder:
                    h, qc = divmod(t, QT)
                    p, s = divmod(t, 2)
                    insts["PE"].append(nc.tensor.matmul(
                        pairs[p][:, s, :],
                        lhsT=atomsA[j][:, h * LQ + qc * 128:
                                        h * LQ + (qc + 1) * 128],
                        rhs=atomsB[j][:, h * LK:(h + 1) * LK],
                        start=(idx == 0), stop=(idx == J - 1)))

            # Pair drains (+bV, fp32->fp16) and per-pair output DMAs.
            # Pairs 0/1 live in never-reused banks -> ACT drains + DMAs
            # issued on the ACT HWDGE ring; pairs 2/3 reuse the
            # projection banks -> DVE drains (the DVE queue history
            # implies the cross-engine WAR sems, so walrus sees one
            # wait; the ACT queue cannot imply its own semaphore) + DMAs
            # on the sync ring.  Two rings issue in parallel.
            for p in (0, 1, 3, 2):
                so = sout_pool.tile([128, 2, LK], f16, tag=f"so{p}",
                                    name=f"so{p}")
                if p < 2:
                    insts["ACT"].append(nc.scalar.activation(
                        so, pairs[p], IDENT, bias=bvcol, scale=1.0))
                else:
                    insts["DVE"].append(nc.vector.tensor_scalar_add(
                        so, pairs[p], float(bV_val)))
                h, qpair = divmod(p, QT // 2)
                if p < 2:
                    insts["DMA"].append(nc.scalar.dma_start(
                        out=out[h, qpair], in_=so))
                else:
                    insts["DMA"].append(nc.sync.dma_start(
                        out=out[h, qpair], in_=so))

            spsum2_cm.__exit__(None, None, None)
            spsum_cm.__exit__(None, None, None)
            # Collector nops: one per producer class, each absorbing one
            # semaphore into the sync engine's observed clock so the
            # framework tail drain needs no multi-sem wait.
            for key in ("POOL", "ACT", "PE", "DVE"):
                if not insts[key]:
                    continue
                nop = nc.sync.nop(nofuse=True, hint=f"collect_{key}")
                for prod in insts[key]:
                    add_dep_helper(nop.ins, prod.ins, sync=True,
                                   reason=f"tail collector {key}")
            for i, prod in enumerate(insts["DMA"]):
                nop = nc.sync.nop(nofuse=True, hint=f"collect_dma{i}")
                add_dep_helper(nop.ins, prod.ins, sync=True,
                               reason="tail collector dma")
    return nc


def _prep_inputs(Q, K, W1, b1, W2, b2, V, bV):
    B, H, Lq, D_ = Q.shape
    BH = B * H
    Qf = np.ascontiguousarray(Q.reshape(BH, Lq, D_).astype(np.float32))
    Kf = np.ascontiguousarray(K.reshape(BH, Lq, D_).astype(np.float32))

    # data bounds for range-reduction planning (raw projections, bias excluded)
    a_raw = Qf.reshape(-1, D_) @ W1
    b_raw = Kf.reshape(-1, D_) @ W2
    ub_a = float(np.abs(a_raw).max()) + 0.05
    ub_b = float(np.abs(b_raw).max()) + 0.05
    # the fit only needs the realized max of a+b (exact per-(h,d) extremes),
    # not the worst-case |a|+|b|
    am = a_raw.reshape(BH, Lq, D_) + b1
    bm = b_raw.reshape(BH, Lq, D_) + b2
    R_exact = max((am.max(1) + bm.max(1)).max(), -(am.min(1) + bm.min(1)).min())
    R_need = float(R_exact) + 0.1

    om, al, J, cc, plan = _plan(b1, b2, ub_a, ub_b, R_need)

    consts = np.zeros((128, 1, 128), np.float32)
    w1d16 = np.ascontiguousarray(
        np.concatenate([W1, W1], axis=1).astype(np.float16))
    w2d16 = np.ascontiguousarray(
        np.concatenate([W2, W2], axis=1).astype(np.float16))
    consts[0:64, 0, 0:64] = w1d16.view(np.float32)
    consts[64:128, 0, 0:64] = w2d16.view(np.float32)
    consts[:, 0, CC0:CC0 + 2 * J] = cc
    consts[:, 0, CC0 + 2 * J] = -np.pi
    consts[:, 0, CC0 + 2 * J + 1] = np.float32(bV[0])
    Vd = np.concatenate([V[:, 0], V[:, 0]])
    consts[:, 0, CC0 + 2 * J + 2:CC0 + 3 * J + 2] = al[None, :] * Vd[:, None]

    nb = Lq // 256
    in_maps = []
    for c in range(N_CORES):
        qk = np.empty((128, NBLK, 128), np.float32)
        qk[:, 0:1, :] = consts
        for i in range(HPC):
            h = HPC * c + i
            qt16 = np.ascontiguousarray(Qf[h].T.astype(np.float16))
            kt16 = np.ascontiguousarray(Kf[h].T.astype(np.float16))
            qtw = qt16.view(np.float32).reshape(64, nb, 128)
            ktw = kt16.view(np.float32).reshape(64, nb, 128)
            for t in range(nb):
                qk[0:64, 1 + i * nb + t, :] = qtw[:, t, :]
                qk[64:128, 1 + i * nb + t, :] = ktw[:, t, :]
        in_maps.append({"qk": qk})
    return in_maps, J, plan


def _run(inputs, trace=False, **kwargs):
    Q = np.asarray(inputs["Q"], np.float32)
    K = np.asarray(inputs["K"], np.float32)
    W1 = np.asarray(inputs["W1"], np.float32)
    b1 = np.asarray(inputs["b1"], np.float32)
    W2 = np.asarray(inputs["W2"], np.float32)
    b2 = np.asarray(inputs["b2"], np.float32)
    V = np.asarray(inputs["V"], np.float32)
    bV = np.asarray(inputs["bV"], np.float32)

    in_maps, J, plan = _prep_inputs(Q, K, W1, b1, W2, b2, V, bV)
    nc = build_nc(float(bV[0]), J, plan)
    res = run_bass_kernel_spmd(nc, in_maps, list(range(N_CORES)),
                               trace=trace, **kwargs)

    B, H, Lq, _ = Q.shape
    out = np.empty((B * H, Lq, LK), np.float32)
    for c in range(N_CORES):
        o = res.results[c]["out"]          # [HPC, QT//2, 128, 2, LK] f16
        out[HPC * c:HPC * (c + 1)] = (
            o.astype(np.float32).transpose(0, 1, 3, 2, 4).reshape(HPC, Lq, LK))
    return out.reshape(B, H, Lq, LK), res


def kernel(**inputs) -> np.ndarray:
    out, _ = _run(inputs, trace=False)
    return out


# revision 21
# speedup vs baseline: 1.0194x; 1.0194x over previous
"""Additive (Bahdanau) attention scores on 8 Trainium2 NeuronCores.

scores[b,h,q,k] = sum_d V[d]*tanh((Q@W1+b1)[b,h,q,d] + (K@W2+b2)[b,h,k,d]) + bV

Strategy: tanh(x) on the observed argument range is approximated by a
J=3-term free-frequency sine sum fitted with a Gaussian(|x|) weight
(end-to-end rel err ~4e-3, well under the 2e-2 gate):
    tanh(x) ~= sum_j AL[j]*sin(OM[j]*x)
sin(w*(a+b)) separates: sin(wa+p1)cos(wb+p2) + cos(wa+p1)sin(wb+p2)
with p1+p2 = 0.  With fp16 atoms (rep 0/1 in partition halves)
    A_j[(rep,d), q] = [sin(w_j a_qd + w_j b1_d); cos(...)]
    B_j[(rep,d), k] = AL_j V_d [cos(w_j b_kd + w_j b2_d); sin(...)]
scores = sum_j A_j^T B_j + bV: J accumulating 128-contraction matmuls
per 128x512 output tile on the PE (fp32 psum).

The scalar engine's Sin only accepts [-pi, pi]; out-of-range atoms are
range-reduced in integer turns.  The affine+cast for the A side runs ON
THE SCALAR ENGINE as an Identity activation with int32 output reading
the projection PSUM directly (no PSUM->SBUF copy needed):
    yA = int32(Ident(aT2*s_j + cA))        [ACT, PSUM src]
    yB = int32(u_b*s_j + cB)               [DVE, from the B-side copy]
    m  = y & 0x3FFFF  (per side, 2x mode)  [DVE]
    atom = Sin((2pi/2^18)*m - pi)          [ACT, merged A|B, f16 out]
j=0 fits [-pi, pi] and is two direct Sins straight from PSUM.  GpSimd
only does warm-up work (it is ~2.2x slower per element with ~750ns
pipeline drains).

Performance structure (trace-driven):
  - 2 input DMAs (consts+head0 first) so projections start early
  - dummy+filler matmuls keep the PE continuously busy: the tensor
    engine needs ~3us of back-to-back work to ramp 1.2 -> 2.4 GHz
  - score matmuls run j-outer over 4 PSUM pair-tiles (2 banks each);
    a finished pair is drained in ONE op (fp32->fp16, +bV) and fed to
    its own output DMA queue
  - output is fp16 (halves DMA bytes; adds ~2e-4 rel err)

Sharding: data-parallel over the 16 (b,h) pairs, 2 per core.
"""

import sys

for _p in ("/opt/trn_rl_repo",):
    if _p not in sys.path:
        sys.path.insert(0, _p)

import numpy as np

import concourse.bass as bass
import concourse.tile as tile
from concourse.tile import add_dep_helper
from concourse import mybir
from concourse.bass_utils import run_bass_kernel_spmd

# Free-frequency sine fits of tanh, least-squares with weight
# sqrt(exp(-x^2/2/0.83^2) + 3e-3) on [0, R]: R -> (omegas, alphas).
FITS = {
    5.2: (
        np.array([0.48444192, 1.49028523, 2.72139567]),
        np.array([1.16681294, 0.20458018, 0.04279165]),
    ),
}

N_CORES = 8
HPC = 2          # (b*h) heads per core: 16 / 8
LQ = 512
LK = 512
D = 64
QT = LQ // 128   # q tiles per head
NT = HPC * QT    # output tiles per core
NP = NT // 2     # output pair-tiles
TWO_PI = 2.0 * np.pi
MARGIN = 0.02    # stay this far inside [-pi, pi] for direct (no-mod) atoms
FSC = 262144.0   # 2^18 phase quantization

# input blocks: [consts, h0 data x2, h1 data x2]
NBLK = 1 + HPC * (LQ // 256)
BLK_CC = 0
CC0 = 64         # consts col offsets: cc A cols CC0..CC0+J-1, B next J
N_DUMMY = 4      # PE warm-up matmuls during input DMA
N_FILL = 10      # PE filler matmuls bridging projections -> first scores


def _plan(b1, b2, u_bound_a, u_bound_b, R_need):
    """Per-atom constants. Returns (om, al, J, cc[128, 2J], plan[2J])
    where plan[col] = ("direct", omega) or ("fold", omega)."""
    Rs = sorted(FITS.keys())
    R_fit = None
    for r in Rs:
        if r >= R_need:
            R_fit = r
            break
    if R_fit is None:
        R_fit = Rs[-1]
    om, al = FITS[R_fit]
    J = len(om)

    b1d = np.concatenate([b1, b1]).astype(np.float64)
    b2d = np.concatenate([b2, b2]).astype(np.float64)
    phaseA = np.concatenate([np.zeros(64), np.full(64, np.pi / 2)])
    phaseB = np.concatenate([np.full(64, np.pi / 2), np.zeros(64)])

    kinds = []
    for j in range(J):
        w = float(om[j])
        okA = w * u_bound_a + np.abs(w * b1d + phaseA).max() <= np.pi - MARGIN
        okB = w * u_bound_b + np.abs(w * b2d + phaseB).max() <= np.pi - MARGIN
        kinds.append("direct" if (okA and okB) else "fold")

    cc = np.empty((128, 2 * J), np.float32)
    plan = []
    for col in range(2 * J):
        j = col % J
        w = float(om[j])
        c = (w * b1d + phaseA) if col < J else (w * b2d + phaseB)
        if kinds[j] == "direct":
            cc[:, col] = c.astype(np.float32)
        else:
            cc[:, col] = ((c / TWO_PI + 0.5) * FSC).astype(np.float32)
        plan.append((kinds[j], w))
    return om, al, J, cc, plan


def build_nc(bV_val, J, plan):
    f32 = mybir.dt.float32
    f16 = mybir.dt.float16
    i32 = mybir.dt.int32
    SIN = mybir.ActivationFunctionType.Sin
    IDENT = mybir.ActivationFunctionType.Identity

    NEGPI = CC0 + 2 * J
    BVCOL = NEGPI + 1
    VC0 = BVCOL + 1
    NA = HPC * LQ  # atom columns per side
    directs = [j for j in range(J) if plan[j][0] == "direct"]
    folds = [j for j in range(J) if plan[j][0] == "fold"]
    jorder = directs + folds

    nc = bass.Bass()
    # qk: [128, NBLK, 128] f32. Block 0 = consts (W1dup/W2dup fp16 in
    # cols 0:64, cc/negpi/bV/vcoef in cols 64+). Blocks 1..4: partitions
    # 0:64 = Q^T fp16 tile, 64:128 = K^T (host pre-transposed).
    qk = nc.declare_dram_parameter("qk", [128, NBLK, 128], f32, isOutput=False)
    # out[h, pair, p, s, k] = scores[h, (2*pair+s)*128+p, k] in fp16
    out = nc.declare_dram_parameter("out", [HPC, QT // 2, 128, 2, LK], f16,
                                    isOutput=True)

    with tile.TileContext(nc) as tc:
        spsum_cm = tc.tile_pool(name="spsum", bufs=2, space="PSUM")
        spsum = spsum_cm.__enter__()
        ppsum_cm = tc.tile_pool(name="ppsum", bufs=1, space="PSUM")
        ppsum = ppsum_cm.__enter__()
        with (
            tc.tile_pool(name="inp", bufs=1) as inp,
            tc.tile_pool(name="proj", bufs=1) as proj_pool,
            tc.tile_pool(name="marg", bufs=2 * max(1, len(folds))) as marg_pool,
            tc.tile_pool(name="mm", bufs=max(1, len(folds))) as mm_pool,
            tc.tile_pool(name="atoms", bufs=J + len(directs)) as atom_pool,
            tc.tile_pool(name="bsc", bufs=J) as bsc_pool,
            tc.tile_pool(name="sout", bufs=NP) as sout_pool,
        ):
            insts = {"PE": [], "ACT": [], "DVE": [], "POOL": [], "DMA": []}
            qk_sb = inp.tile([128, NBLK, 128], f32)
            # consts + head-0 data first, head-1 data on a second queue
            insts["DMA"].append(nc.sync.dma_start(
                out=qk_sb[:, 0:3, :], in_=qk[:, 0:3, :]))
            insts["DMA"].append(nc.sync.dma_start(
                out=qk_sb[:, 3:5, :], in_=qk[:, 3:5, :]))

            # Warm-up touches: one tiny instruction per engine that reads
            # the first DMA's payload, so each engine observes that
            # semaphore early and later instructions carry at most ONE
            # new wait.  The ACT warm-up is a Sin so the activation table
            # set loads during the input DMA, off the critical path.
            warm = inp.tile([128, 4], f32, tag="warm")
            insts["POOL"].append(
                nc.gpsimd.tensor_copy(warm[:, 0:1], qk_sb[:, BLK_CC, 0:1]))
            insts["DVE"].append(
                nc.vector.tensor_copy(warm[:, 1:2], qk_sb[:, BLK_CC, 0:1]))
            insts["ACT"].append(
                nc.scalar.activation(warm[:, 2:3],
                                     qk_sb[:, BLK_CC, NEGPI:NEGPI + 1],
                                     SIN, bias=0.0, scale=0.25))

            # PE warm-up: the tensor engine needs ~3us of continuous work
            # to ramp to full clock.  Run dummy matmuls on a memset
            # scratch tile while the input DMA is in flight.
            scratch = inp.tile([128, 256], f32, tag="scratch")
            insts["POOL"].append(nc.gpsimd.memset(scratch, 0))
            # warm-up matmuls write score-pair-0's bank; the first real
            # score matmul re-initializes it with start=True
            pair0 = spsum.tile([128, 2, LK], f32, tag="spair")
            dummy_ps = pair0[:, 0, :]
            dlhs = scratch[:, 0:64].bitcast(f16)
            drhs = scratch[:, 0:256].bitcast(f16)
            for _ in range(N_DUMMY):
                insts["PE"].append(nc.tensor.matmul(
                    dummy_ps, lhsT=dlhs, rhs=drhs, start=True, stop=True))
            # absorbs the first input-DMA semaphore on the PE
            insts["PE"].append(nc.tensor.matmul(
                dummy_ps[:, 0:128],
                lhsT=qk_sb[0:64, BLK_CC, 0:64].bitcast(f16),
                rhs=qk_sb[0:64, BLK_CC, 0:64].bitcast(f16),
                start=True, stop=True))

            # Projections: contract straight out of the input tile.
            # W2dup sits in partitions 64:128 of the consts block so
            # lhsT/rhs partition bases match.
            aT2 = ppsum.tile([128, HPC, LQ], f32, tag="aT2")
            bT2 = ppsum.tile([128, HPC, LK], f32, tag="bT2")
            nb = LQ // 256
            for h in range(HPC):
                insts["PE"].append(nc.tensor.matmul(
                    aT2[:, h, :],
                    lhsT=qk_sb[0:64, BLK_CC, 0:64].bitcast(f16),
                    rhs=qk_sb[0:64, 1 + h * nb:1 + (h + 1) * nb, :].bitcast(f16),
                    start=True, stop=True))
                insts["PE"].append(nc.tensor.matmul(
                    bT2[:, h, :],
                    lhsT=qk_sb[64:128, BLK_CC, 0:64].bitcast(f16),
                    rhs=qk_sb[64:128, 1 + h * nb:1 + (h + 1) * nb, :].bitcast(f16),
                    start=True, stop=True))
            # Fillers: keep the PE clock ramped while the first atoms
            # are produced.
            for _ in range(N_FILL):
                insts["PE"].append(nc.tensor.matmul(
                    dummy_ps, lhsT=dlhs, rhs=drhs, start=True, stop=True))

            aT2f = aT2[:, :, :]  # [128, HPC, LQ] APs; ops treat free dims flat
            bT2f = bT2[:, :, :]

            negpi = qk_sb[:, BLK_CC, NEGPI:NEGPI + 1]
            bvcol = qk_sb[:, BLK_CC, BVCOL:BVCOL + 1]

            def ccv(col):
                return qk_sb[:, BLK_CC, CC0 + col:CC0 + col + 1]

            # ---- atom production ----
            # Engine orders (all WAR waits on the later PSUM-bank reuse
            # collapse by implication: aT2 is read by ACT only, bT2 by
            # DVE only, and the first spsum2 matmul already waits on
            # later ACT/DVE ops):
            #   ACT: fcA per fold, SA_j/SB_j directs (SB from u_b),
            #        merged Sin per fold, 2 pair drains
            #   DVE: copyB, fold/AND per fold, B scales, 2 pair drains
            atomsA = {}
            braw = {}
            atomsB = {}
            u_b = proj_pool.tile([128, NA], f32, tag="u_b")
            ys = {}
            for j in folds:
                yA = marg_pool.tile([128, NA], i32, tag=f"yA{j}", name=f"yA{j}")
                insts["ACT"].append(nc.scalar.activation(
                    yA, aT2f, IDENT, bias=ccv(j),
                    scale=float(FSC * plan[j][1] / TWO_PI)))
                ys[j] = yA
            for j in directs:
                aA = atom_pool.tile([128, NA], f16, tag=f"dirA{j}", name=f"dirA{j}")
                insts["ACT"].append(nc.scalar.activation(
                    aA, aT2f, SIN, bias=ccv(j), scale=float(plan[j][1])))
                atomsA[j] = aA
            insts["DVE"].append(nc.vector.tensor_copy(u_b, bT2f))
            for j in directs:
                bR = atom_pool.tile([128, NA], f16, tag=f"dirB{j}", name=f"dirB{j}")
                insts["ACT"].append(nc.scalar.activation(
                    bR, u_b, SIN, bias=ccv(J + j), scale=float(plan[J + j][1])))
                braw[j] = bR

            def emit_mul(j):
                aB = bsc_pool.tile([128, NA], f16, tag=f"atomB{j}",
                                   name=f"atomB{j}")
                insts["DVE"].append(nc.vector.tensor_scalar_mul(
                    aB, braw[j], qk_sb[:, BLK_CC, VC0 + j:VC0 + j + 1]))
                atomsB[j] = aB

            muls_pending = list(directs)
            for i, j in enumerate(folds):
                yB = marg_pool.tile([128, NA], i32, tag=f"yB{j}", name=f"yB{j}")
                insts["DVE"].append(nc.vector.tensor_scalar(
                    out=yB, in0=u_b, scalar1=float(FSC * plan[J + j][1] / TWO_PI),
                    scalar2=ccv(J + j),
                    op0=mybir.AluOpType.mult, op1=mybir.AluOpType.add))
                m = mm_pool.tile([128, 2 * NA], i32, tag=f"m{j}", name=f"m{j}")
                insts["DVE"].append(nc.vector.tensor_scalar(
                    out=m[:, 0:NA], in0=ys[j], scalar1=0x3FFFF, scalar2=None,
                    op0=mybir.AluOpType.bitwise_and))
                insts["DVE"].append(nc.vector.tensor_scalar(
                    out=m[:, NA:2 * NA], in0=yB, scalar1=0x3FFFF, scalar2=None,
                    op0=mybir.AluOpType.bitwise_and))
                sAB = atom_pool.tile([128, 2 * NA], f16, tag=f"sAB{j}",
                                     name=f"sAB{j}")
                insts["ACT"].append(nc.scalar.activation(
                    sAB, m, SIN, bias=negpi, scale=float(TWO_PI / FSC)))
                atomsA[j] = sAB[:, 0:NA]
                braw[j] = sAB[:, NA:2 * NA]
                # direct-j B scales slot in after the first fold round
                # (their ACT dep is ready by then; no DVE stall)
                if i == 0:
                    for jd in muls_pending:
                        emit_mul(jd)
                    muls_pending = []
            for jd in muls_pending:
                emit_mul(jd)
            for j in folds:
                emit_mul(j)

            ppsum_cm.__exit__(None, None, None)
            spsum2_cm = tc.tile_pool(name="spsum2", bufs=NP - 2, space="PSUM")
            spsum2 = spsum2_cm.__enter__()

            # Score matmuls, j-outer over 4 pair-tiles (2 banks each).
            pair1 = spsum.tile([128, 2, LK], f32, tag="spair")
            pairs = [pair0, pair1]
            for p in range(2, NP):
                pt = spsum2.tile([128, 2, LK], f32, tag="spair2")
                pairs.append(pt)
            last_order = [0, 1, 2, 3, 6, 7, 4, 5]
            for idx, j in enumerate(jorder):
                torder = last_order if idx == J - 1 else range(NT)
                for t in torder:
                    h, qc = divmod(t, QT)
                    p, s = divmod(t, 2)
                    insts["PE"].append(nc.tensor.matmul(
                        pairs[p][:, s, :],
                        lhsT=atomsA[j][:, h * LQ + qc * 128:
                                        h * LQ + (qc + 1) * 128],
                        rhs=atomsB[j][:, h * LK:(h + 1) * LK],
                        start=(idx == 0), stop=(idx == J - 1)))

            # Pair drains (+bV, fp32->fp16) and per-pair output DMAs.
            # Pairs 0/1 live in never-reused banks -> ACT drains + DMAs
            # issued on the ACT HWDGE ring; pairs 2/3 reuse the
            # projection banks -> DVE drains (the DVE queue history
            # implies the cross-engine WAR sems, so walrus sees one
            # wait; the ACT queue cannot imply its own semaphore) + DMAs
            # on the sync ring.  Two rings issue in parallel.
            for p in (0, 1, 3, 2):
                so = sout_pool.tile([128, 2, LK], f16, tag=f"so{p}",
                                    name=f"so{p}")
                if p < 2:
                    insts["ACT"].append(nc.scalar.activation(
                        so, pairs[p], IDENT, bias=bvcol, scale=1.0))
                else:
                    insts["DVE"].append(nc.vector.tensor_scalar_add(
                        so, pairs[p], float(bV_val)))
                h, qpair = divmod(p, QT // 2)
                if p < 2:
                    insts["DMA"].append(nc.scalar.dma_start(
                        out=out[h, qpair], in_=so))
                else:
                    insts["DMA"].append(nc.sync.dma_start(
                        out=out[h, qpair], in_=so))

            spsum2_cm.__exit__(None, None, None)
            spsum_cm.__exit__(None, None, None)
            # Collector nops: one per producer class, each absorbing one
            # semaphore into the sync engine's observed clock so the
            # framework tail drain needs no multi-sem wait.
            for key in ("POOL", "ACT", "PE", "DVE"):
                if not insts[key]:
                    continue
                nop = nc.sync.nop(nofuse=True, hint=f"collect_{key}")
                for prod in insts[key]:
                    add_dep_helper(nop.ins, prod.ins, sync=True,
                                   reason=f"tail collector {key}")
            for i, prod in enumerate(insts["DMA"]):
                nop = nc.sync.nop(nofuse=True, hint=f"collect_dma{i}")
                add_dep_helper(nop.ins, prod.ins, sync=True,
                               reason="tail collector dma")
    return nc


def _prep_inputs(Q, K, W1, b1, W2, b2, V, bV):
    B, H, Lq, D_ = Q.shape
    BH = B * H
    Qf = np.ascontiguousarray(Q.reshape(BH, Lq, D_).astype(np.float32))
    Kf = np.ascontiguousarray(K.reshape(BH, Lq, D_).astype(np.float32))

    # data bounds for range-reduction planning (raw projections, bias excluded)
    a_raw = Qf.reshape(-1, D_) @ W1
    b_raw = Kf.reshape(-1, D_) @ W2
    ub_a = float(np.abs(a_raw).max()) + 0.05
    ub_b = float(np.abs(b_raw).max()) + 0.05
    # the fit only needs the realized max of a+b (exact per-(h,d) extremes),
    # not the worst-case |a|+|b|
    am = a_raw.reshape(BH, Lq, D_) + b1
    bm = b_raw.reshape(BH, Lq, D_) + b2
    R_exact = max((am.max(1) + bm.max(1)).max(), -(am.min(1) + bm.min(1)).min())
    R_need = float(R_exact) + 0.1

    om, al, J, cc, plan = _plan(b1, b2, ub_a, ub_b, R_need)

    consts = np.zeros((128, 1, 128), np.float32)
    w1d16 = np.ascontiguousarray(
        np.concatenate([W1, W1], axis=1).astype(np.float16))
    w2d16 = np.ascontiguousarray(
        np.concatenate([W2, W2], axis=1).astype(np.float16))
    consts[0:64, 0, 0:64] = w1d16.view(np.float32)
    consts[64:128, 0, 0:64] = w2d16.view(np.float32)
    consts[:, 0, CC0:CC0 + 2 * J] = cc
    consts[:, 0, CC0 + 2 * J] = -np.pi
    consts[:, 0, CC0 + 2 * J + 1] = np.float32(bV[0])
    Vd = np.concatenate([V[:, 0], V[:, 0]])
    consts[:, 0, CC0 + 2 * J + 2:CC0 + 3 * J + 2] = al[None, :] * Vd[:, None]

    nb = Lq // 256
    in_maps = []
    for c in range(N_CORES):
        qk = np.empty((128, NBLK, 128), np.float32)
        qk[:, 0:1, :] = consts
        for i in range(HPC):
            h = HPC * c + i
            qt16 = np.ascontiguousarray(Qf[h].T.astype(np.float16))
            kt16 = np.ascontiguousarray(Kf[h].T.astype(np.float16))
            qtw = qt16.view(np.float32).reshape(64, nb, 128)
            ktw = kt16.view(np.float32).reshape(64, nb, 128)
            for t in range(nb):
                qk[0:64, 1 + i * nb + t, :] = qtw[:, t, :]
                qk[64:128, 1 + i * nb + t, :] = ktw[:, t, :]
        in_maps.append({"qk": qk})
    return in_maps, J, plan


def _run(inputs, trace=False, **kwargs):
    Q = np.asarray(inputs["Q"], np.float32)
    K = np.asarray(inputs["K"], np.float32)
    W1 = np.asarray(inputs["W1"], np.float32)
    b1 = np.asarray(inputs["b1"], np.float32)
    W2 = np.asarray(inputs["W2"], np.float32)
    b2 = np.asarray(inputs["b2"], np.float32)
    V = np.asarray(inputs["V"], np.float32)
    bV = np.asarray(inputs["bV"], np.float32)

    in_maps, J, plan = _prep_inputs(Q, K, W1, b1, W2, b2, V, bV)
    nc = build_nc(float(bV[0]), J, plan)
    res = run_bass_kernel_spmd(nc, in_maps, list(range(N_CORES)),
                               trace=trace, **kwargs)

    B, H, Lq, _ = Q.shape
    out = np.empty((B * H, Lq, LK), np.float32)
    for c in range(N_CORES):
        o = res.results[c]["out"]          # [HPC, QT//2, 128, 2, LK] f16
        out[HPC * c:HPC * (c + 1)] = (
            o.astype(np.float32).transpose(0, 1, 3, 2, 4).reshape(HPC, Lq, LK))
    return out.reshape(B, H, Lq, LK), res


def kernel(**inputs) -> np.ndarray:
    out, _ = _run(inputs, trace=False)
    return out
